# revision 13
# baseline (speedup 1.0000x reference)
"""Causal multi-head attention (QKV projection + softmax(QK^T)V) on 8 TRN2 NeuronCores.

Problem: x[4,2048,1024] @ W_qkv[1024,3072] + b_qkv -> 16-head causal attention -> [4,2048,1024].

Sharding: core i = (batch bi=i//2, head-group hg=i%2). Each core handles 1 batch x 8 heads,
fully data/tensor-parallel (no collectives). Host pre-arranges per-core inputs (all matmul
operands fp16; accumulation f32 in PSUM):
  - x passed pre-transposed [1024, 2048] so the contraction dim lands on partitions with
    plain contiguous DMAs (no on-device transposes anywhere).
  - wqk [1024,1024] pair-major (pair p: Q cols at 256p, K at 256p+128), head-PAIR-stacked
    (64+64 rows) so QKV^T matmul output chunks are directly the [hd, n] stacked layout the
    attention stage consumes.
  - wv [1024,520]: V columns with per-head stride 65; col 65h+64 is a zero column, and
    the replicated bias tile bv has 1.0 there, so the "ones column" that makes the PV
    matmul accumulate softmax denominators (and b_v itself) ride the DVE PSUM->SBUF
    drain as a tensor_add -- no bias matmuls at all.
Device pipeline per core:
  QKV^T matmuls (Q^T pair-stacked, K^T pair-packed: head h occupies rows 64*(h%2) of its
  pair's slab, S^T runs as a 64-contraction matmul at base_partition 64*(h%2), which also
  row-tiles the two heads onto disjoint PE array row-groups) -> S^T = K Q^T per key-chunk
  with causal column trimming -> one ScalarE Exp(scale=1/8) per 2-chunk group, PSUM->SBUF
  fp16 = P^T -> causal tri-mask multiply on the 128x128 diagonal blocks only (DVE) ->
  PV matmuls accumulate [q, 64 cols + denominator] per q-block (both heads packed in one
  PSUM bank) -> reciprocal (DVE) * scale (DVE late / ScalarE early) epilogue ->
  DMA out [2048, 512] f32.
Scheduling: ScalarE exp (~160us) and TensorE (~190us) must overlap near-perfectly.
 - A dozen warm-up matmuls on memset scratch run first so the PE HAM clock-gate reaches
   K=8/8 (2.4 GHz) before real work; without them the DMA-gated trickle start keeps the
   PE at 1.2 GHz for the first ~20us of real matmuls.
 - Input DMA is spread over three rings so triggers issue in parallel: sync ring carries
   the latency-critical x-stripe-0 + wqk-pair-0 pieces per-chunk; the scalar ring (idle
   before the first softmax) carries batched x stripes 1-3; gpsimd SWDGE carries the
   rest (wqk pairs 1-3, wv, biases) -- ScalarE's instruction stream stays pure exps by
   the time softmax starts.
 - Attention runs stripe-major across head-pairs (t-major rounds). QKV matmul tiles are
   distributed across blocks by a build-time reverse-greedy quota scheduler: each block's
   spare PE time (ACT cost minus S^T/PV cost) is filled latest-first subject to each
   tile's consumer deadline, so the ACT-heavy late rounds keep the PE dense instead of
   exhausting the filler early.
 - Each stripe's PV matmuls are deferred into the next block's S^T/exp loop (inlined
   per-diagonal-chunk for the final block so the tail doesn't serialize).
"""

import numpy as np

import concourse.bass as bass
import concourse.tile as tile
from concourse import bacc, mybir
from concourse import bass_utils

F16 = mybir.dt.float16
F32 = mybir.dt.float32

B, N, D = 4, 2048, 1024
H = 16  # global heads
HD = 64
HL = 8  # heads per core
N_CORES = 8
P = 128
NT = N // P  # 16 token tiles
KC = D // P  # 8 contraction chunks
VW = HL * (HD + 1)  # 520
VH = VW // 2  # 260

_cache = {}


def _build():
    nc = bacc.Bacc("TRN2", target_bir_lowering=False, debug=False)

    x_d = nc.dram_tensor("x", [D, N], F16, kind="ExternalInput").ap()  # x^T, host-transposed
    wqk_d = nc.dram_tensor("wqk", [D, 1024], F16, kind="ExternalInput").ap()
    wv_d = nc.dram_tensor("wv", [D, VW], F16, kind="ExternalInput").ap()
    bqk_d = nc.dram_tensor("bqk", [P, 8], F32, kind="ExternalInput").ap()
    bv_d = nc.dram_tensor("bv", [P, VW], F16, kind="ExternalInput").ap()
    tri_d = nc.dram_tensor("tri", [P, P], F16, kind="ExternalInput").ap()
    out_d = nc.dram_tensor("out", [N, HL * HD], F32, kind="ExternalOutput").ap()

    wqk_r = wqk_d.rearrange("(k p) n -> p k n", p=P)
    wv_r = wv_d.rearrange("(k p) n -> p k n", p=P)
    x_r = x_d.rearrange("(k p) n -> p k n", p=P)

    with tile.TileContext(nc) as tc:
        with (
            tc.tile_pool(name="const", bufs=1) as cpool,
            tc.tile_pool(name="pt", bufs=2) as ptpool,
            tc.tile_pool(name="opair", bufs=6) as oppool,
            tc.tile_pool(name="misc", bufs=6) as mpool,
            tc.tile_pool(name="ps_mm", bufs=2, space="PSUM") as ps_mm,
            tc.tile_pool(name="ps_s", bufs=2, space="PSUM") as ps_s,
            tc.tile_pool(name="ps_o", bufs=2, space="PSUM") as ps_o,
        ):
            # ---- constants / inputs to SBUF ----
            xt_sb = cpool.tile([P, KC, N], F16, name="xt_sb")  # x^T, 8 chunks of [128, 2048]
            wqk_sb = cpool.tile([P, KC, 1024], F16, name="wqk_sb")
            wv_sb = cpool.tile([P, KC, VW], F16, name="wv_sb")
            bqk_sb = cpool.tile([P, 8], F32, name="bqk_sb")
            bv_sb = cpool.tile([P, VW], F16, name="bv_sb")  # b_v (+ones col) replicated
            tri_sb = cpool.tile([P, P], F16, name="tri_sb")
            qt_sb = cpool.tile([P, 4, N], F16, name="qt_sb")  # Q^T pair-stacked
            # K^T pair-packed: pair pr's slab holds head 2pr K^T in rows 0:64 and head
            # 2pr+1 in rows 64:128. S^T runs as a 64-row contraction at base_partition
            # 64*hh -- no zero padding, no memset, and the two heads land on disjoint
            # PE row-groups so their LDWEIGHTS/MATMULs overlap in the array.
            kt_sb = cpool.tile([P, 4, N], F16, name="kt_sb")
            v_sb = cpool.tile([P, NT, VW], F16, name="v_sb")
            wu_sb = cpool.tile([P, 640], F16, name="wu_sb")  # PE warm-up scratch

            # PE warm-up: the HAM clock gate un-throttles (1.2 -> 2.4 GHz) only after
            # ~3.4us of sustained PE activity. Burn that window on scratch matmuls while
            # the input DMAs are in flight, so real matmuls start at full clock.
            nc.gpsimd.memset(wu_sb[:], 0.0)
            psw = ps_mm.tile([P, 512], F32, tag="mm", name="ps_warm")
            for i in range(6):
                nc.tensor.matmul(
                    psw[:],
                    lhsT=wu_sb[:, 0:P],
                    rhs=wu_sb[:, P : P + 512],
                    start=(i == 0),
                    stop=(i == 5),
                )

            # Preload the exp table set (~2.7us) before the first real softmax exp, so
            # it doesn't pay ACT_TABLE_LOAD.
            warm = mpool.tile([1, 8], F32, tag="warm", name="warm")
            nc.gpsimd.memset(warm[:], 0.0)
            nc.scalar.activation(warm[:], warm[:], mybir.ActivationFunctionType.Exp)

            # ---- input DMA ----
            # The start is DMA-bandwidth-bound (all 8 cores pull inputs at once), so
            # the critical first-tile bytes (x stripe-0 + wqk pair-0, 1.5 MB) go out
            # per-chunk on TWO rings in parallel -- x pieces on sync, wqk pieces on
            # the scalar HWDGE ring (idle until the first exp) -- letting the first
            # QKV chains track chunk arrivals. Bulk follows on sync in deadline order.
            nc.sync.dma_start(bqk_sb[:], bqk_d)
            for k in range(KC):
                nc.sync.dma_start(xt_sb[:, k, 0:512], x_d[k * P : (k + 1) * P, 0:512])
                nc.scalar.dma_start(wqk_sb[:, k, 0:256], wqk_r[:, k, 0:256])
            nc.sync.dma_start(tri_sb[:], tri_d)
            nc.sync.dma_start(wv_sb[:, :, :], wv_r[:, :, :])
            nc.sync.dma_start(wqk_sb[:, :, 256:512], wqk_r[:, :, 256:512])
            nc.sync.dma_start(bv_sb[:], bv_d)
            nc.sync.dma_start(xt_sb[:, :, 512:1024], x_r[:, :, 512:1024])
            nc.sync.dma_start(wqk_sb[:, :, 512:768], wqk_r[:, :, 512:768])
            nc.sync.dma_start(wqk_sb[:, :, 768:1024], wqk_r[:, :, 768:1024])
            nc.sync.dma_start(xt_sb[:, :, 1024:1536], x_r[:, :, 1024:1536])
            nc.sync.dma_start(xt_sb[:, :, 1536:2048], x_r[:, :, 1536:2048])

            done_qk = set()  # (c, tt, part)
            done_v = set()  # (j, half, part)
            open_mm = {}  # half-open QKV psum tiles

            # QKV tiles are emitted in two 4-chunk parts so the per-group hook work
            # stays under ~1us -- a bigger PE lump between S^T groups delays the next
            # group's matmuls (engine FIFO) past ScalarE's exp cadence and the slip
            # accumulates. Parts of one tile are kept adjacent so at most one QKV
            # psum tile is half-open (ps_mm has 2 bufs).
            def emit_qk_part(c, tt, part):
                if (c, tt, part) in done_qk:
                    return
                if part == 1 and (c, tt, 0) not in done_qk:
                    emit_qk_part(c, tt, 0)
                done_qk.add((c, tt, part))
                pr = c % 4
                if part == 0:
                    pq = ps_mm.tile([P, 512], F32, tag="mm", name=f"pq_{c}_{tt}")
                    open_mm[("qk", c, tt)] = pq
                else:
                    pq = open_mm.pop(("qk", c, tt))
                col0 = 256 * (c % 4) + (0 if c < 4 else 128)
                for k in range(4 * part, 4 * part + 4):
                    nc.tensor.matmul(
                        pq[:],
                        lhsT=wqk_sb[:, k, col0 : col0 + P],
                        rhs=xt_sb[:, k, tt * 512 : (tt + 1) * 512],
                        start=(k == 0),
                        stop=(k == KC - 1),
                    )
                if part == 0:
                    return

                def badd(out, in_, b):
                    nc.vector.tensor_scalar_add(out, in_, b)

                if c < 4:
                    badd(
                        qt_sb[:, pr, tt * 512 : (tt + 1) * 512], pq[:], bqk_sb[:, c : c + 1]
                    )
                else:
                    for hh in (0, 1):
                        rows = slice(64 * hh, 64 * hh + 64)
                        badd(
                            kt_sb[rows, pr, tt * 512 : (tt + 1) * 512],
                            pq[rows, :],
                            bqk_sb[rows, c : c + 1],
                        )

            def emit_qk(c, tt):
                emit_qk_part(c, tt, 0)
                emit_qk_part(c, tt, 1)

            def emit_v_part(j, half, part):
                if (j, half, part) in done_v:
                    return
                if part == 1 and (j, half, 0) not in done_v:
                    emit_v_part(j, half, 0)
                done_v.add((j, half, part))
                if part == 0:
                    pv = ps_mm.tile([P, VH], F32, tag="mm", name=f"pv_{j}_{half}")
                    open_mm[("v", j, half)] = pv
                else:
                    pv = open_mm.pop(("v", j, half))
                for k in range(4 * part, 4 * part + 4):
                    nc.tensor.matmul(
                        pv[:],
                        lhsT=xt_sb[:, k, j * P : (j + 1) * P],
                        rhs=wv_sb[:, k, half * VH : (half + 1) * VH],
                        start=(k == 0),
                        stop=(k == KC - 1),
                    )
                if part == 0:
                    return
                # bias (and the denominator ones-column) ride the PSUM->SBUF drain
                nc.vector.tensor_add(
                    v_sb[:, j, half * VH : (half + 1) * VH],
                    pv[:],
                    bv_sb[:, half * VH : (half + 1) * VH],
                )

            def emit_v(j, half):
                emit_v_part(j, half, 0)
                emit_v_part(j, half, 1)

            # ---- build-time quota scheduler for QKV filler ----
            # blocks processed t-major: n = 4*t + p
            n_blocks = 16

            def act_cost(t):
                return (2 * t + 1) * 2086 + 1400

            def pe_s_cost(t):
                # the two heads' S^T matmuls run concurrently (disjoint row groups),
                # so per-group wall is the single-head column count
                return (2048 * t + 1280) * 0.4167 + (2 * t + 2) * 120

            def pv_cost(t):
                return 2 * (16 * t + 10) * 34.0

            QK_COST = 8 * 512 * 0.4167 + 8 * 15
            V_COST = 8 * 260 * 0.4167 + 8 * 15

            # items: (kind, a, b, deadline_block, earliest_block, cost)
            items = []
            for tt in range(4):
                for pr in range(4):
                    if (pr, tt) == (0, 0):
                        continue  # prologue
                    dl = 4 * tt + pr
                    for c in (pr, 4 + pr):
                        items.append(["qk", c, tt, dl, 2 * tt, QK_COST])
            for tj in range(4):
                for j in range(4 * tj, 4 * tj + 4):
                    for half in (0, 1):
                        dl = 4 * tj + 2 * half + 1
                        items.append(["v", j, half, dl, 2 * tj, V_COST])

            cap = []
            for n in range(n_blocks):
                t = n // 4
                c = act_cost(t) - pe_s_cost(t)
                if n >= 1:
                    c -= pv_cost((n - 1) // 4)
                if n == n_blocks - 1:
                    c -= pv_cost(3)  # own inline PV
                cap.append(max(0.0, c))

            # Forward greedy: place items as EARLY as capacity allows (ACT has its
            # structural slack in the small early rounds; late rounds must run with
            # ScalarE saturated and no extra PE work between S^T groups).
            W = [[] for _ in range(n_blocks)]
            remaining = list(items)
            for n in range(n_blocks):
                room = cap[n]
                # mandatory: last chance for items with deadline n+1
                musts = [it for it in remaining if it[3] == n + 1]
                for it in musts:
                    W[n].append(it)
                    room -= it[5]
                    remaining.remove(it)
                elig = [it for it in remaining if it[4] <= n]
                elig.sort(key=lambda it: it[3])  # earliest deadline first
                for it in elig:
                    if room <= 0:
                        break
                    W[n].append(it)
                    room -= it[5]
                    remaining.remove(it)
            for n in range(n_blocks):
                W[n].sort(key=lambda it: it[3])  # urgent first within a block

            def item_units(it):
                if it[0] == "qk":
                    return [("qk", it[1], it[2], 0), ("qk", it[1], it[2], 1)]
                return [("v", it[1], it[2], 0), ("v", it[1], it[2], 1)]

            def emit_unit(u):
                if u[0] == "qk":
                    emit_qk_part(u[1], u[2], u[3])
                else:
                    emit_v_part(u[1], u[2], u[3])

            state = {}

            def emit_pv_half(p, t, pt, r, hh, ctx, split_dma=False):
                """One head's PV chain for q-block i = 4t+r; epilogue+DMA after hh=1.
                Both heads' accumulators share one PSUM bank ([128, 2, 65])."""
                i = 4 * t + r
                hl = 2 * p + hh
                for j in range(i + 1):  # safety: deps normally already emitted
                    emit_v(j, hl // 4)
                if hh == 0:
                    ctx["opair"] = oppool.tile([P, P], F32, tag="op", name=f"op_{p}_{i}")
                    ctx["po"] = po = ps_o.tile([P, 2, 65], F32, tag="o", name=f"po_{p}_{i}")
                else:
                    po = ctx["po"]
                for j in range(i + 1):
                    nc.tensor.matmul(
                        po[:, hh, :],
                        lhsT=pt[:, hh, j, r * P : (r + 1) * P],
                        rhs=v_sb[:, j, 65 * hl : 65 * hl + 65],
                        start=(j == 0),
                        stop=(j == i),
                    )
                if hh == 0:
                    return
                opair = ctx["opair"]
                rc = mpool.tile([P, 2], F32, tag="rc", name=f"rc_{p}_{i}")
                nc.vector.reciprocal(rc[:], po[:, :, 64])
                for h2 in (0, 1):
                    if state.get("pos", 0) == 0:
                        # early rounds are PE/DVE-bound and ScalarE has slack
                        nc.scalar.mul(
                            opair[:, 64 * h2 : 64 * h2 + 64],
                            po[:, h2, 0:64],
                            rc[:, h2 : h2 + 1],
                        )
                    else:
                        nc.vector.tensor_scalar_mul(
                            opair[:, 64 * h2 : 64 * h2 + 64], po[:, h2, 0:64], rc[:, h2 : h2 + 1]
                        )
                    if split_dma:
                        nc.sync.dma_start(
                            out_d[i * P : (i + 1) * P, p * P + 64 * h2 : p * P + 64 * h2 + 64],
                            opair[:, 64 * h2 : 64 * h2 + 64],
                        )
                if not split_dma:
                    nc.sync.dma_start(out_d[i * P : (i + 1) * P, p * P : (p + 1) * P], opair[:])

            def emit_pv(p, t, pt, r, split_dma=False):
                ctx = {}
                emit_pv_half(p, t, pt, r, 0, ctx, split_dma)
                emit_pv_half(p, t, pt, r, 1, ctx, split_dma)

            # Prologue: the first block's own QK tiles, k-chunks interleaved across
            # both tiles so the pair completes as soon as the last chunk's DMA lands.
            done_qk.update({(0, 0, 0), (0, 0, 1), (4, 0, 0), (4, 0, 1)})
            pq0 = ps_mm.tile([P, 512], F32, tag="mm", name="pq_pro_0")
            pq4 = ps_mm.tile([P, 512], F32, tag="mm", name="pq_pro_4")
            for k in range(KC):
                for c, pq in ((0, pq0), (4, pq4)):
                    col0 = 0 if c < 4 else 128
                    nc.tensor.matmul(
                        pq[:],
                        lhsT=wqk_sb[:, k, col0 : col0 + P],
                        rhs=xt_sb[:, k, 0:512],
                        start=(k == 0),
                        stop=(k == KC - 1),
                    )
            nc.vector.tensor_scalar_add(qt_sb[:, 0, 0:512], pq0[:], bqk_sb[:, 0:1])
            for hh in (0, 1):
                rows = slice(64 * hh, 64 * hh + 64)
                nc.vector.tensor_scalar_add(
                    kt_sb[rows, 0, 0:512], pq4[rows, :], bqk_sb[rows, 4:5]
                )

            pv_queue = []  # per-half pop units: (p, t, pt, r, hh, ctx)
            blocks = [(pos, t, p) for pos, t in enumerate((0, 1, 2, 3)) for p in range(4)]
            for n, (pos, t, p) in enumerate(blocks):
                state["pos"] = pos
                last = n == len(blocks) - 1
                for tt in range(t + 1):  # safety: deps normally already emitted
                    emit_qk(p, tt)
                    emit_qk(4 + p, tt)
                units = []
                for it in W[n]:
                    units.extend(item_units(it))

                def group_hooks(units=units):
                    if pv_queue:
                        emit_pv_half(*pv_queue.pop(0))
                    if units:
                        emit_unit(units.pop(0))

                # pt layout: [128, hh, chunk, 512]
                pt = ptpool.tile([P, 2, 16, 512], F16, tag="pt", name=f"pt_{p}_{t}")

                # S^T + exp in groups of 2 chunks per head; diagonal chunks only
                # compute the causal-valid columns (stale psum prefix is bounded
                # old scores: exp'd then never consumed).
                for g in range(2 * t + 2):
                    psA = ps_s.tile([P, 2, 512], F32, tag="s", name=f"psA_{p}_{t}_{g}")
                    psB = ps_s.tile([P, 2, 512], F32, tag="s", name=f"psB_{p}_{t}_{g}")
                    for jj in (0, 1):
                        j = 2 * g + jj
                        q0 = 128 * (j - 4 * t) if j >= 4 * t else 0
                        for hh, ps in ((0, psA), (1, psB)):
                            nc.tensor.matmul(
                                ps[:, jj, q0:512],
                                lhsT=kt_sb[64 * hh : 64 * hh + 64, p, j * P : (j + 1) * P],
                                rhs=qt_sb[
                                    64 * hh : 64 * hh + 64,
                                    p,
                                    t * 512 + q0 : (t + 1) * 512,
                                ],
                                start=True,
                                stop=True,
                            )
                    for hh, ps in ((0, psA), (1, psB)):
                        if g == 2 * t + 1:
                            # fully-diagonal group: one act over both chunks'
                            # 256:512 suffix (chunk 2g+1's 256:384 is stale psum
                            # -- finite old scores, exp'd but never consumed --
                            # cheaper than a second act instruction)
                            nc.scalar.activation(
                                pt[:, hh, 2 * g : 2 * g + 2, 256:512],
                                ps[:, :, 256:512],
                                mybir.ActivationFunctionType.Exp,
                                scale=0.125,
                            )
                        else:
                            nc.scalar.activation(
                                pt[:, hh, 2 * g : 2 * g + 2, :],
                                ps[:],
                                mybir.ActivationFunctionType.Exp,
                                scale=0.125,
                            )
                    group_hooks()
                    if last and g >= 2 * t:
                        # final block: mask + PV inline per diagonal pair so the
                        # tail doesn't serialize after the last exp
                        for r in (0, 1) if g == 2 * t else (2, 3):
                            j = 4 * t + r
                            for hh in (0, 1):
                                blk = pt[:, hh, j, r * P : (r + 1) * P]
                                # DVE here: this mask sits on the tail critical
                                # chain and DVE is ~3x faster than GpSimd
                                nc.vector.tensor_mul(blk, blk, tri_sb[:])
                            emit_pv(p, t, pt, r, split_dma=(g == 2 * t + 1))
                while units:
                    emit_unit(units.pop(0))
                while pv_queue:
                    emit_pv_half(*pv_queue.pop(0))
                if last:
                    continue
                # causal mask on diagonal 128x128 blocks (DVE: ~3x faster than
                # GpSimd and it has slack; next block's PV pops need these early)
                for hh in (0, 1):
                    for r in range(4):
                        j = 4 * t + r
                        blk = pt[:, hh, j, r * P : (r + 1) * P]
                        nc.vector.tensor_mul(blk, blk, tri_sb[:])
                pv_queue = []
                for r in range(4):
                    ctx = {}
                    pv_queue.append((p, t, pt, r, 0, ctx))
                    pv_queue.append((p, t, pt, r, 1, ctx))
            while pv_queue:
                emit_pv_half(*pv_queue.pop(0))

    nc.compile()
    return nc


def get_nc():
    if "nc" not in _cache:
        _cache["nc"] = _build()
    return _cache["nc"]


def _prep_core_inputs(x, W, b, bi, hg):
    h0 = hg * HL
    Wq = W[:, 0:D].reshape(D, H, HD)
    Wk = W[:, D : 2 * D].reshape(D, H, HD)
    Wv = W[:, 2 * D :].reshape(D, H, HD)
    bq = b[0:D].reshape(H, HD)
    bk = b[D : 2 * D].reshape(H, HD)
    bv = b[2 * D :].reshape(H, HD)

    # pair-major: pair p occupies cols [256p, 256p+256) as [Q pair | K pair]
    wqk = np.empty((D, 1024), np.float32)
    bqk = np.empty((P, 8), np.float32)
    for c in range(4):
        for half in range(2):
            h = h0 + 2 * c + half
            sl = slice(256 * c + half * HD, 256 * c + half * HD + HD)
            wqk[:, sl] = Wq[:, h]
            bqk[half * HD : (half + 1) * HD, c] = bq[h]
            sl = slice(256 * c + P + half * HD, 256 * c + P + half * HD + HD)
            wqk[:, sl] = Wk[:, h]
            bqk[half * HD : (half + 1) * HD, 4 + c] = bk[h]

    wv_aug = np.zeros((D, VW), np.float32)
    bv_aug = np.zeros((VW,), np.float32)
    for hl in range(HL):
        wv_aug[:, 65 * hl : 65 * hl + HD] = Wv[:, h0 + hl]
        bv_aug[65 * hl : 65 * hl + HD] = bv[h0 + hl]
        bv_aug[65 * hl + HD] = 1.0

    tri = np.triu(np.ones((P, P), np.float32))  # tri[k, q] = 1 where q >= k

    return {
        "x": np.ascontiguousarray(x[bi].astype(np.float16).T),
        "wqk": wqk.astype(np.float16),
        "wv": wv_aug.astype(np.float16),
        "bqk": bqk,
        "bv": np.broadcast_to(bv_aug.astype(np.float16), (P, VW)).copy(),
        "tri": tri.astype(np.float16),
    }


def make_in_maps(x, W_qkv, b_qkv):
    x = np.asarray(x, dtype=np.float32)
    W = np.asarray(W_qkv, dtype=np.float32)
    b = np.asarray(b_qkv, dtype=np.float32)
    return [_prep_core_inputs(x, W, b, i // 2, i % 2) for i in range(N_CORES)]


def assemble(results):
    out = np.empty((B, N, D), np.float32)
    for i in range(N_CORES):
        bi, hg = i // 2, i % 2
        out[bi, :, hg * 512 : (hg + 1) * 512] = results[i]["out"]
    return out


def run(x, W_qkv, b_qkv, trace=False, tmpdir=None):
    nc = get_nc()
    in_maps = make_in_maps(x, W_qkv, b_qkv)
    res = bass_utils.run_bass_kernel_spmd(
        nc, in_maps, core_ids=list(range(N_CORES)), trace=trace, tmpdir=tmpdir
    )
    return assemble(res.results), res


def kernel(x, W_qkv, b_qkv):
    out, _ = run(x, W_qkv, b_qkv)
    return out


# revision 16
# speedup vs baseline: 1.0125x; 1.0125x over previous
"""Causal multi-head attention (QKV projection + softmax(QK^T)V) on 8 TRN2 NeuronCores.

Problem: x[4,2048,1024] @ W_qkv[1024,3072] + b_qkv -> 16-head causal attention -> [4,2048,1024].

Sharding: core i = (batch bi=i//2, head-group hg=i%2). Each core handles 1 batch x 8 heads,
fully data/tensor-parallel (no collectives). Host pre-arranges per-core inputs (all matmul
operands fp16; accumulation f32 in PSUM):
  - x passed pre-transposed [1024, 2048] so the contraction dim lands on partitions with
    plain contiguous DMAs (no on-device transposes anywhere).
  - wqk [1024,1024] pair-major (pair p: Q cols at 256p, K at 256p+128), head-PAIR-stacked
    (64+64 rows) so QKV^T matmul output chunks are directly the [hd, n] stacked layout the
    attention stage consumes.
  - wv [1024,520]: V columns with per-head stride 65; col 65h+64 is a zero column, and
    the replicated bias tile bv has 1.0 there, so the "ones column" that makes the PV
    matmul accumulate softmax denominators (and b_v itself) ride the DVE PSUM->SBUF
    drain as a tensor_add -- no bias matmuls at all.
Device pipeline per core:
  QKV^T matmuls (Q^T pair-stacked, K^T pair-packed: head h occupies rows 64*(h%2) of its
  pair's slab, S^T runs as a 64-contraction matmul at base_partition 64*(h%2), which also
  row-tiles the two heads onto disjoint PE array row-groups) -> S^T = K Q^T per key-chunk
  with causal column trimming -> one ScalarE Exp(scale=1/8) per 2-chunk group, PSUM->SBUF
  fp16 = P^T -> causal tri-mask multiply on the 128x128 diagonal blocks only (DVE) ->
  PV matmuls accumulate [q, 64 cols + denominator] per q-block (both heads packed in one
  PSUM bank) -> reciprocal (DVE) * scale (DVE late / ScalarE early) epilogue ->
  DMA out [2048, 512] f32.
Scheduling: ScalarE exp (~160us) and TensorE (~190us) must overlap near-perfectly.
 - A dozen warm-up matmuls on memset scratch run first so the PE HAM clock-gate reaches
   K=8/8 (2.4 GHz) before real work; without them the DMA-gated trickle start keeps the
   PE at 1.2 GHz for the first ~20us of real matmuls.
 - Input DMA is spread over three rings so triggers issue in parallel: sync ring carries
   the latency-critical x-stripe-0 + wqk-pair-0 pieces per-chunk; the scalar ring (idle
   before the first softmax) carries batched x stripes 1-3; gpsimd SWDGE carries the
   rest (wqk pairs 1-3, wv, biases) -- ScalarE's instruction stream stays pure exps by
   the time softmax starts.
 - Attention runs stripe-major across head-pairs (t-major rounds). QKV matmul tiles are
   distributed across blocks by a build-time reverse-greedy quota scheduler: each block's
   spare PE time (ACT cost minus S^T/PV cost) is filled latest-first subject to each
   tile's consumer deadline, so the ACT-heavy late rounds keep the PE dense instead of
   exhausting the filler early.
 - Each stripe's PV matmuls are deferred into the next block's S^T/exp loop (inlined
   per-diagonal-chunk for the final block so the tail doesn't serialize).
"""

import numpy as np

import concourse.bass as bass
import concourse.tile as tile
from concourse import bacc, mybir
from concourse import bass_utils

F16 = mybir.dt.float16
F32 = mybir.dt.float32

B, N, D = 4, 2048, 1024
H = 16  # global heads
HD = 64
HL = 8  # heads per core
N_CORES = 8
P = 128
NT = N // P  # 16 token tiles
KC = D // P  # 8 contraction chunks
VW = HL * (HD + 1)  # 520
VH = VW // 2  # 260

_cache = {}


def _build():
    nc = bacc.Bacc("TRN2", target_bir_lowering=False, debug=False)

    x_d = nc.dram_tensor("x", [D, N], F16, kind="ExternalInput").ap()  # x^T, host-transposed
    wqk_d = nc.dram_tensor("wqk", [D, 1024], F16, kind="ExternalInput").ap()
    wv_d = nc.dram_tensor("wv", [D, VW], F16, kind="ExternalInput").ap()
    bqk_d = nc.dram_tensor("bqk", [P, 8], F32, kind="ExternalInput").ap()
    bv_d = nc.dram_tensor("bv", [P, VW], F16, kind="ExternalInput").ap()
    tri_d = nc.dram_tensor("tri", [P, P], F16, kind="ExternalInput").ap()
    out_d = nc.dram_tensor("out", [N, HL * HD], F32, kind="ExternalOutput").ap()

    wqk_r = wqk_d.rearrange("(k p) n -> p k n", p=P)
    wv_r = wv_d.rearrange("(k p) n -> p k n", p=P)
    x_r = x_d.rearrange("(k p) n -> p k n", p=P)

    with tile.TileContext(nc) as tc:
        with (
            tc.tile_pool(name="const", bufs=1) as cpool,
            tc.tile_pool(name="pt", bufs=2) as ptpool,
            tc.tile_pool(name="opair", bufs=6) as oppool,
            tc.tile_pool(name="misc", bufs=6) as mpool,
            tc.tile_pool(name="ps_mm", bufs=2, space="PSUM") as ps_mm,
            tc.tile_pool(name="ps_s", bufs=2, space="PSUM") as ps_s,
            tc.tile_pool(name="ps_o", bufs=2, space="PSUM") as ps_o,
        ):
            # ---- constants / inputs to SBUF ----
            xt_sb = cpool.tile([P, KC, N], F16, name="xt_sb")  # x^T, 8 chunks of [128, 2048]
            wqk_sb = cpool.tile([P, KC, 1024], F16, name="wqk_sb")
            wv_sb = cpool.tile([P, KC, VW], F16, name="wv_sb")
            bqk_sb = cpool.tile([P, 8], F32, name="bqk_sb")
            bv_sb = cpool.tile([P, VW], F16, name="bv_sb")  # b_v (+ones col) replicated
            tri_sb = cpool.tile([P, P], F16, name="tri_sb")
            qt_sb = cpool.tile([P, 4, N], F16, name="qt_sb")  # Q^T pair-stacked
            # K^T pair-packed: pair pr's slab holds head 2pr K^T in rows 0:64 and head
            # 2pr+1 in rows 64:128. S^T runs as a 64-row contraction at base_partition
            # 64*hh -- no zero padding, no memset, and the two heads land on disjoint
            # PE row-groups so their LDWEIGHTS/MATMULs overlap in the array.
            kt_sb = cpool.tile([P, 4, N], F16, name="kt_sb")
            v_sb = cpool.tile([P, NT, VW], F16, name="v_sb")
            wu_sb = cpool.tile([P, 640], F16, name="wu_sb")  # PE warm-up scratch

            # PE warm-up: the HAM clock gate un-throttles (1.2 -> 2.4 GHz) only after
            # ~3.4us of sustained PE activity. Burn that window on scratch matmuls while
            # the input DMAs are in flight, so real matmuls start at full clock.
            nc.gpsimd.memset(wu_sb[:], 0.0)
            psw = ps_mm.tile([P, 512], F32, tag="mm", name="ps_warm")
            for i in range(12):
                nc.tensor.matmul(
                    psw[:],
                    lhsT=wu_sb[:, 0:P],
                    rhs=wu_sb[:, P : P + 512],
                    start=(i == 0),
                    stop=(i == 11),
                )

            # Preload the exp table set (~2.7us) before the first real softmax exp, so
            # it doesn't pay ACT_TABLE_LOAD.
            warm = mpool.tile([1, 8], F32, tag="warm", name="warm")
            nc.gpsimd.memset(warm[:], 0.0)
            nc.scalar.activation(warm[:], warm[:], mybir.ActivationFunctionType.Exp)

            # ---- input DMA: one ring (sync), batched, ordered by first-use deadline ----
            # The start is DMA-bandwidth-bound (all 8 cores pull inputs at once);
            # batched triggers amortize the per-trigger latency and the single ring
            # keeps the critical first-tile bytes (x stripe-0 + wqk pair-0) ahead of
            # the bulk in the DMA-engine queues.
            nc.sync.dma_start(bqk_sb[:], bqk_d)
            nc.sync.dma_start(xt_sb[:, 0:4, 0:512], x_r[:, 0:4, 0:512])
            nc.sync.dma_start(wqk_sb[:, 0:4, 0:256], wqk_r[:, 0:4, 0:256])
            nc.sync.dma_start(xt_sb[:, 4:8, 0:512], x_r[:, 4:8, 0:512])
            nc.sync.dma_start(wqk_sb[:, 4:8, 0:256], wqk_r[:, 4:8, 0:256])
            nc.sync.dma_start(tri_sb[:], tri_d)
            nc.sync.dma_start(wv_sb[:, :, :], wv_r[:, :, :])
            nc.sync.dma_start(wqk_sb[:, :, 256:512], wqk_r[:, :, 256:512])
            nc.sync.dma_start(bv_sb[:], bv_d)
            nc.sync.dma_start(xt_sb[:, :, 512:1024], x_r[:, :, 512:1024])
            nc.sync.dma_start(wqk_sb[:, :, 512:768], wqk_r[:, :, 512:768])
            nc.sync.dma_start(wqk_sb[:, :, 768:1024], wqk_r[:, :, 768:1024])
            nc.sync.dma_start(xt_sb[:, :, 1024:1536], x_r[:, :, 1024:1536])
            nc.sync.dma_start(xt_sb[:, :, 1536:2048], x_r[:, :, 1536:2048])

            done_qk = set()  # (c, tt, part)
            done_v = set()  # (j, half, part)
            open_mm = {}  # half-open QKV psum tiles

            # QKV tiles are emitted in two 4-chunk parts so the per-group hook work
            # stays under ~1us -- a bigger PE lump between S^T groups delays the next
            # group's matmuls (engine FIFO) past ScalarE's exp cadence and the slip
            # accumulates. Parts of one tile are kept adjacent so at most one QKV
            # psum tile is half-open (ps_mm has 2 bufs).
            def emit_qk_part(c, tt, part):
                if (c, tt, part) in done_qk:
                    return
                if part == 1 and (c, tt, 0) not in done_qk:
                    emit_qk_part(c, tt, 0)
                done_qk.add((c, tt, part))
                pr = c % 4
                if part == 0:
                    pq = ps_mm.tile([P, 512], F32, tag="mm", name=f"pq_{c}_{tt}")
                    open_mm[("qk", c, tt)] = pq
                else:
                    pq = open_mm.pop(("qk", c, tt))
                col0 = 256 * (c % 4) + (0 if c < 4 else 128)
                for k in range(4 * part, 4 * part + 4):
                    nc.tensor.matmul(
                        pq[:],
                        lhsT=wqk_sb[:, k, col0 : col0 + P],
                        rhs=xt_sb[:, k, tt * 512 : (tt + 1) * 512],
                        start=(k == 0),
                        stop=(k == KC - 1),
                    )
                if part == 0:
                    return

                def badd(out, in_, b):
                    nc.vector.tensor_scalar_add(out, in_, b)

                if c < 4:
                    badd(
                        qt_sb[:, pr, tt * 512 : (tt + 1) * 512], pq[:], bqk_sb[:, c : c + 1]
                    )
                else:
                    for hh in (0, 1):
                        rows = slice(64 * hh, 64 * hh + 64)
                        badd(
                            kt_sb[rows, pr, tt * 512 : (tt + 1) * 512],
                            pq[rows, :],
                            bqk_sb[rows, c : c + 1],
                        )

            def emit_qk(c, tt):
                emit_qk_part(c, tt, 0)
                emit_qk_part(c, tt, 1)

            def emit_v_part(j, half, part):
                if (j, half, part) in done_v:
                    return
                if part == 1 and (j, half, 0) not in done_v:
                    emit_v_part(j, half, 0)
                done_v.add((j, half, part))
                if part == 0:
                    pv = ps_mm.tile([P, VH], F32, tag="mm", name=f"pv_{j}_{half}")
                    open_mm[("v", j, half)] = pv
                else:
                    pv = open_mm.pop(("v", j, half))
                for k in range(4 * part, 4 * part + 4):
                    nc.tensor.matmul(
                        pv[:],
                        lhsT=xt_sb[:, k, j * P : (j + 1) * P],
                        rhs=wv_sb[:, k, half * VH : (half + 1) * VH],
                        start=(k == 0),
                        stop=(k == KC - 1),
                    )
                if part == 0:
                    return
                # bias (and the denominator ones-column) ride the PSUM->SBUF drain
                nc.vector.tensor_add(
                    v_sb[:, j, half * VH : (half + 1) * VH],
                    pv[:],
                    bv_sb[:, half * VH : (half + 1) * VH],
                )

            def emit_v(j, half):
                emit_v_part(j, half, 0)
                emit_v_part(j, half, 1)

            # ---- build-time quota scheduler for QKV filler ----
            # blocks processed t-major: n = 4*t + p
            n_blocks = 16

            def act_cost(t):
                return (2 * t + 1) * 2086 + 1400

            def pe_s_cost(t):
                # the two heads' S^T matmuls run concurrently (disjoint row groups),
                # so per-group wall is the single-head column count
                return (2048 * t + 1280) * 0.4167 + (2 * t + 2) * 120

            def pv_cost(t):
                return 2 * (16 * t + 10) * 34.0

            QK_COST = 8 * 512 * 0.4167 + 8 * 15
            V_COST = 8 * 260 * 0.4167 + 8 * 15

            # items: (kind, a, b, deadline_block, earliest_block, cost)
            items = []
            for tt in range(4):
                for pr in range(4):
                    if (pr, tt) == (0, 0):
                        continue  # prologue
                    dl = 4 * tt + pr
                    for c in (pr, 4 + pr):
                        items.append(["qk", c, tt, dl, 2 * tt, QK_COST])
            for tj in range(4):
                for j in range(4 * tj, 4 * tj + 4):
                    for half in (0, 1):
                        dl = 4 * tj + 2 * half + 1
                        items.append(["v", j, half, dl, 2 * tj, V_COST])

            cap = []
            for n in range(n_blocks):
                t = n // 4
                c = act_cost(t) - pe_s_cost(t)
                if n >= 1:
                    c -= pv_cost((n - 1) // 4)
                if n == n_blocks - 1:
                    c -= pv_cost(3)  # own inline PV
                cap.append(max(0.0, c))

            # Forward greedy: place items as EARLY as capacity allows (ACT has its
            # structural slack in the small early rounds; late rounds must run with
            # ScalarE saturated and no extra PE work between S^T groups).
            W = [[] for _ in range(n_blocks)]
            remaining = list(items)
            for n in range(n_blocks):
                room = cap[n]
                # mandatory: last chance for items with deadline n+1
                musts = [it for it in remaining if it[3] == n + 1]
                for it in musts:
                    W[n].append(it)
                    room -= it[5]
                    remaining.remove(it)
                elig = [it for it in remaining if it[4] <= n]
                elig.sort(key=lambda it: it[3])  # earliest deadline first
                for it in elig:
                    if room <= 0:
                        break
                    W[n].append(it)
                    room -= it[5]
                    remaining.remove(it)
            for n in range(n_blocks):
                W[n].sort(key=lambda it: it[3])  # urgent first within a block

            def item_units(it):
                if it[0] == "qk":
                    return [("qk", it[1], it[2], 0), ("qk", it[1], it[2], 1)]
                return [("v", it[1], it[2], 0), ("v", it[1], it[2], 1)]

            def emit_unit(u):
                if u[0] == "qk":
                    emit_qk_part(u[1], u[2], u[3])
                else:
                    emit_v_part(u[1], u[2], u[3])

            state = {}

            def emit_pv_half(p, t, pt, r, hh, ctx, split_dma=False):
                """One head's PV chain for q-block i = 4t+r; epilogue+DMA after hh=1.
                Both heads' accumulators share one PSUM bank ([128, 2, 65])."""
                i = 4 * t + r
                hl = 2 * p + hh
                for j in range(i + 1):  # safety: deps normally already emitted
                    emit_v(j, hl // 4)
                if hh == 0:
                    ctx["opair"] = oppool.tile([P, P], F32, tag="op", name=f"op_{p}_{i}")
                    ctx["po"] = po = ps_o.tile([P, 2, 65], F32, tag="o", name=f"po_{p}_{i}")
                else:
                    po = ctx["po"]
                for j in range(i + 1):
                    nc.tensor.matmul(
                        po[:, hh, :],
                        lhsT=pt[:, hh, j, r * P : (r + 1) * P],
                        rhs=v_sb[:, j, 65 * hl : 65 * hl + 65],
                        start=(j == 0),
                        stop=(j == i),
                    )
                if hh == 0:
                    return
                opair = ctx["opair"]
                rc = mpool.tile([P, 2], F32, tag="rc", name=f"rc_{p}_{i}")
                nc.vector.reciprocal(rc[:], po[:, :, 64])
                for h2 in (0, 1):
                    if state.get("pos", 0) == 0:
                        # early rounds are PE/DVE-bound and ScalarE has slack
                        nc.scalar.mul(
                            opair[:, 64 * h2 : 64 * h2 + 64],
                            po[:, h2, 0:64],
                            rc[:, h2 : h2 + 1],
                        )
                    else:
                        nc.vector.tensor_scalar_mul(
                            opair[:, 64 * h2 : 64 * h2 + 64], po[:, h2, 0:64], rc[:, h2 : h2 + 1]
                        )
                    if split_dma:
                        nc.sync.dma_start(
                            out_d[i * P : (i + 1) * P, p * P + 64 * h2 : p * P + 64 * h2 + 64],
                            opair[:, 64 * h2 : 64 * h2 + 64],
                        )
                if not split_dma:
                    nc.sync.dma_start(out_d[i * P : (i + 1) * P, p * P : (p + 1) * P], opair[:])

            def emit_pv(p, t, pt, r, split_dma=False):
                ctx = {}
                emit_pv_half(p, t, pt, r, 0, ctx, split_dma)
                emit_pv_half(p, t, pt, r, 1, ctx, split_dma)

            # Prologue: the first block's own QK tiles so S^T (0,0) can start ASAP.
            emit_qk(0, 0)
            emit_qk(4, 0)

            pv_queue = []  # per-half pop units: (p, t, pt, r, hh, ctx)
            blocks = [(pos, t, p) for pos, t in enumerate((0, 1, 2, 3)) for p in range(4)]
            for n, (pos, t, p) in enumerate(blocks):
                state["pos"] = pos
                last = n == len(blocks) - 1
                for tt in range(t + 1):  # safety: deps normally already emitted
                    emit_qk(p, tt)
                    emit_qk(4 + p, tt)
                units = []
                for it in W[n]:
                    units.extend(item_units(it))

                def group_hooks(units=units):
                    if pv_queue:
                        emit_pv_half(*pv_queue.pop(0))
                    if units:
                        emit_unit(units.pop(0))

                # pt layout: [128, hh, chunk, 512]
                pt = ptpool.tile([P, 2, 16, 512], F16, tag="pt", name=f"pt_{p}_{t}")

                # S^T + exp in groups of 2 chunks per head; diagonal chunks only
                # compute the causal-valid columns (stale psum prefix is bounded
                # old scores: exp'd then never consumed).
                for g in range(2 * t + 2):
                    psA = ps_s.tile([P, 2, 512], F32, tag="s", name=f"psA_{p}_{t}_{g}")
                    psB = ps_s.tile([P, 2, 512], F32, tag="s", name=f"psB_{p}_{t}_{g}")
                    for jj in (0, 1):
                        j = 2 * g + jj
                        q0 = 128 * (j - 4 * t) if j >= 4 * t else 0
                        for hh, ps in ((0, psA), (1, psB)):
                            nc.tensor.matmul(
                                ps[:, jj, q0:512],
                                lhsT=kt_sb[64 * hh : 64 * hh + 64, p, j * P : (j + 1) * P],
                                rhs=qt_sb[
                                    64 * hh : 64 * hh + 64,
                                    p,
                                    t * 512 + q0 : (t + 1) * 512,
                                ],
                                start=True,
                                stop=True,
                            )
                    for hh, ps in ((0, psA), (1, psB)):
                        if g == 2 * t + 1:
                            # fully-diagonal group: one act over both chunks'
                            # 256:512 suffix (chunk 2g+1's 256:384 is stale psum
                            # -- finite old scores, exp'd but never consumed --
                            # cheaper than a second act instruction)
                            nc.scalar.activation(
                                pt[:, hh, 2 * g : 2 * g + 2, 256:512],
                                ps[:, :, 256:512],
                                mybir.ActivationFunctionType.Exp,
                                scale=0.125,
                            )
                        else:
                            nc.scalar.activation(
                                pt[:, hh, 2 * g : 2 * g + 2, :],
                                ps[:],
                                mybir.ActivationFunctionType.Exp,
                                scale=0.125,
                            )
                    group_hooks()
                    if last and g >= 2 * t:
                        # final block: mask + PV inline per diagonal pair so the
                        # tail doesn't serialize after the last exp
                        for r in (0, 1) if g == 2 * t else (2, 3):
                            j = 4 * t + r
                            for hh in (0, 1):
                                blk = pt[:, hh, j, r * P : (r + 1) * P]
                                # DVE here: this mask sits on the tail critical
                                # chain and DVE is ~3x faster than GpSimd
                                nc.vector.tensor_mul(blk, blk, tri_sb[:])
                            emit_pv(p, t, pt, r, split_dma=(g == 2 * t + 1))
                while units:
                    emit_unit(units.pop(0))
                while pv_queue:
                    emit_pv_half(*pv_queue.pop(0))
                if last:
                    continue
                # causal mask on diagonal 128x128 blocks (DVE: ~3x faster than
                # GpSimd and it has slack; next block's PV pops need these early)
                for hh in (0, 1):
                    for r in range(4):
                        j = 4 * t + r
                        blk = pt[:, hh, j, r * P : (r + 1) * P]
                        nc.vector.tensor_mul(blk, blk, tri_sb[:])
                pv_queue = []
                for r in range(4):
                    ctx = {}
                    pv_queue.append((p, t, pt, r, 0, ctx))
                    pv_queue.append((p, t, pt, r, 1, ctx))
            while pv_queue:
                emit_pv_half(*pv_queue.pop(0))

    nc.compile()
    return nc


def get_nc():
    if "nc" not in _cache:
        _cache["nc"] = _build()
    return _cache["nc"]


def _prep_core_inputs(x, W, b, bi, hg):
    h0 = hg * HL
    Wq = W[:, 0:D].reshape(D, H, HD)
    Wk = W[:, D : 2 * D].reshape(D, H, HD)
    Wv = W[:, 2 * D :].reshape(D, H, HD)
    bq = b[0:D].reshape(H, HD)
    bk = b[D : 2 * D].reshape(H, HD)
    bv = b[2 * D :].reshape(H, HD)

    # pair-major: pair p occupies cols [256p, 256p+256) as [Q pair | K pair]
    wqk = np.empty((D, 1024), np.float32)
    bqk = np.empty((P, 8), np.float32)
    for c in range(4):
        for half in range(2):
            h = h0 + 2 * c + half
            sl = slice(256 * c + half * HD, 256 * c + half * HD + HD)
            wqk[:, sl] = Wq[:, h]
            bqk[half * HD : (half + 1) * HD, c] = bq[h]
            sl = slice(256 * c + P + half * HD, 256 * c + P + half * HD + HD)
            wqk[:, sl] = Wk[:, h]
            bqk[half * HD : (half + 1) * HD, 4 + c] = bk[h]

    wv_aug = np.zeros((D, VW), np.float32)
    bv_aug = np.zeros((VW,), np.float32)
    for hl in range(HL):
        wv_aug[:, 65 * hl : 65 * hl + HD] = Wv[:, h0 + hl]
        bv_aug[65 * hl : 65 * hl + HD] = bv[h0 + hl]
        bv_aug[65 * hl + HD] = 1.0

    tri = np.triu(np.ones((P, P), np.float32))  # tri[k, q] = 1 where q >= k

    return {
        "x": np.ascontiguousarray(x[bi].astype(np.float16).T),
        "wqk": wqk.astype(np.float16),
        "wv": wv_aug.astype(np.float16),
        "bqk": bqk,
        "bv": np.broadcast_to(bv_aug.astype(np.float16), (P, VW)).copy(),
        "tri": tri.astype(np.float16),
    }


def make_in_maps(x, W_qkv, b_qkv):
    x = np.asarray(x, dtype=np.float32)
    W = np.asarray(W_qkv, dtype=np.float32)
    b = np.asarray(b_qkv, dtype=np.float32)
    return [_prep_core_inputs(x, W, b, i // 2, i % 2) for i in range(N_CORES)]


def assemble(results):
    out = np.empty((B, N, D), np.float32)
    for i in range(N_CORES):
        bi, hg = i // 2, i % 2
        out[bi, :, hg * 512 : (hg + 1) * 512] = results[i]["out"]
    return out


def run(x, W_qkv, b_qkv, trace=False, tmpdir=None):
    nc = get_nc()
    in_maps = make_in_maps(x, W_qkv, b_qkv)
    res = bass_utils.run_bass_kernel_spmd(
        nc, in_maps, core_ids=list(range(N_CORES)), trace=trace, tmpdir=tmpdir
    )
    return assemble(res.results), res


def kernel(x, W_qkv, b_qkv):
    out, _ = run(x, W_qkv, b_qkv)
    return out


# revision 26
# speedup vs baseline: 1.1738x; 1.1593x over previous
"""Causal multi-head attention (QKV projection + softmax(QK^T)V) on 8 TRN2 NeuronCores.

Problem: x[4,2048,1024] @ W_qkv[1024,3072] + b_qkv -> 16-head causal attention -> [4,2048,1024].

Sharding: core i = (batch bi=i//2, head-group hg=i%2). Each core handles 1 batch x 8 heads,
fully data/tensor-parallel (no collectives). Host pre-arranges per-core inputs (all matmul
operands fp16; accumulation f32 in PSUM):
  - x passed pre-transposed [1024, 2048] so the contraction dim lands on partitions with
    plain contiguous DMAs (no on-device transposes anywhere).
  - wqk [1024,1024] pair-major (pair p: Q cols at 256p, K at 256p+128), head-PAIR-stacked
    (64+64 rows) so QKV^T matmul output chunks are directly the [hd, n] stacked layout the
    attention stage consumes.
  - wv [1024,520]: V columns with per-head stride 65; col 65h+64 is a zero column, and
    the replicated bias tile bv has 1.0 there, so the "ones column" that makes the PV
    matmul accumulate softmax denominators (and b_v itself) ride the DVE PSUM->SBUF
    drain as a tensor_add -- no bias matmuls at all.
Device pipeline per core:
  QKV^T matmuls (Q^T pair-stacked, K^T zero-padded per head so S^T runs K=128 with fast
  weight load -- a 64-row contraction measures ~1.8x slower per matmul, FWL needs the
  full 128 rows) -> S^T = K Q^T per key-chunk with causal column trimming -> one ScalarE
  Exp(scale=1/8) per 2-chunk group (the fully-diagonal group merged into a single act
  over both chunks' 256:512 suffix; the stale sliver is never consumed), PSUM->SBUF
  fp16 = P^T -> causal tri-mask multiply on the 128x128 diagonal blocks only (DVE) ->
  PV matmuls accumulate [q, 64 cols + denominator] per q-block (both heads packed in one
  PSUM bank) -> reciprocal (DVE) * scale (DVE late / ScalarE early) epilogue ->
  DMA out [2048, 512] f32.
Scheduling: ScalarE exp (~163us) and TensorE (~191us) must overlap near-perfectly.
 - A dozen warm-up matmuls on memset scratch run first so the PE HAM clock-gate reaches
   K=8/8 (2.4 GHz) before real work; without them the DMA-gated start keeps the PE at
   1.2 GHz for the first ~20us of real matmuls (HAM needs ~3.4us of sustained activity).
 - Input DMA is batched on the sync ring in first-use-deadline order (the start is
   DMA-bandwidth-bound with all 8 cores pulling at once; big triggers amortize the
   ~620ns trigger serialization): bqk, then x-stripe-0/wqk-pair-0 as two 4-chunk
   batches each (the prologue QK chains are emitted in matching k 0-3 / 4-7 halves),
   then the bulk.
 - kt's zero half-slabs are cleared by 8 per-slab memsets split across DVE (slabs 0-1)
   and gpsimd (slabs 2-7) in slab order, so the first S^T unblocks ~10.5us in instead
   of being gated by a monolithic 13.7us gpsimd memset.
 - Attention runs stripe-major across head-pairs (t-major rounds); QKV matmul tiles are
   a deadline-ordered "filler" queue drained between S^T groups (next block's QK tiles
   are prefetched from the current block's hooks); each stripe's PV matmuls are
   deferred into the next block's S^T/exp loop. PV pops stay whole and early in the
   block so the previous pt tile is consumed before the next block's first exp rotates
   the pt pool. The final block inlines mask+PV per diagonal chunk and pushes its
   output DMAs onto the idle scalar HWDGE ring so the tail doesn't serialize on the
   sync ring's DMA-sem slots.
"""

import numpy as np

import concourse.bass as bass
import concourse.tile as tile
from concourse import bacc, mybir
from concourse import bass_utils

F16 = mybir.dt.float16
F32 = mybir.dt.float32

B, N, D = 4, 2048, 1024
H = 16  # global heads
HD = 64
HL = 8  # heads per core
N_CORES = 8
P = 128
NT = N // P  # 16 token tiles
KC = D // P  # 8 contraction chunks
VW = HL * (HD + 1)  # 520
VH = VW // 2  # 260

_cache = {}


def _build():
    nc = bacc.Bacc("TRN2", target_bir_lowering=False, debug=False)

    x_d = nc.dram_tensor("x", [D, N], F16, kind="ExternalInput").ap()  # x^T, host-transposed
    wqk_d = nc.dram_tensor("wqk", [D, 1024], F16, kind="ExternalInput").ap()
    wv_d = nc.dram_tensor("wv", [D, VW], F16, kind="ExternalInput").ap()
    bqk_d = nc.dram_tensor("bqk", [P, 8], F32, kind="ExternalInput").ap()
    bv_d = nc.dram_tensor("bv", [P, VW], F16, kind="ExternalInput").ap()
    tri_d = nc.dram_tensor("tri", [P, P], F16, kind="ExternalInput").ap()
    out_d = nc.dram_tensor("out", [N, HL * HD], F32, kind="ExternalOutput").ap()

    wqk_r = wqk_d.rearrange("(k p) n -> p k n", p=P)
    wv_r = wv_d.rearrange("(k p) n -> p k n", p=P)

    with tile.TileContext(nc) as tc:
        with (
            tc.tile_pool(name="const", bufs=1) as cpool,
            tc.tile_pool(name="pt", bufs=2) as ptpool,
            tc.tile_pool(name="opair", bufs=6) as oppool,
            tc.tile_pool(name="misc", bufs=6) as mpool,
            tc.tile_pool(name="ps_mm", bufs=2, space="PSUM") as ps_mm,
            tc.tile_pool(name="ps_s", bufs=2, space="PSUM") as ps_s,
            tc.tile_pool(name="ps_o", bufs=2, space="PSUM") as ps_o,
        ):
            # ---- constants / inputs to SBUF ----
            xt_sb = cpool.tile([P, KC, N], F16, name="xt_sb")  # x^T, 8 chunks of [128, 2048]
            wqk_sb = cpool.tile([P, KC, 1024], F16, name="wqk_sb")
            wv_sb = cpool.tile([P, KC, VW], F16, name="wv_sb")
            bqk_sb = cpool.tile([P, 8], F32, name="bqk_sb")
            bv_sb = cpool.tile([P, VW], F16, name="bv_sb")  # b_v (+ones col) replicated
            tri_sb = cpool.tile([P, P], F16, name="tri_sb")
            qt_sb = cpool.tile([P, 4, N], F16, name="qt_sb")  # Q^T pair-stacked
            kt_sb = cpool.tile([P, HL, N], F16, name="kt_sb")
            v_sb = cpool.tile([P, NT, VW], F16, name="v_sb")

            wu_sb = cpool.tile([P, 640], F16, name="wu_sb")  # PE warm-up scratch

            # PE warm-up: the HAM clock gate un-throttles (1.2 -> 2.4 GHz) only after
            # ~3.4us of sustained PE activity; burn that window on scratch matmuls
            # while the input DMAs are in flight.
            nc.gpsimd.memset(wu_sb[:], 0.0)
            psw = ps_mm.tile([P, 512], F32, tag="mm", name="ps_warm")
            for i in range(12):
                nc.tensor.matmul(
                    psw[:],
                    lhsT=wu_sb[:, 0:P],
                    rhs=wu_sb[:, P : P + 512],
                    start=(i == 0),
                    stop=(i == 11),
                )

            warm = mpool.tile([1, 8], F32, tag="warm", name="warm")
            nc.gpsimd.memset(warm[:], 0.0)
            nc.scalar.activation(warm[:], warm[:], mybir.ActivationFunctionType.Exp)
            # kt zero-padding memset, split across DVE + gpsimd in slab order so the
            # first S^T (pair 0 -> slabs 0,1) unblocks ~10.5us in instead of ~23:
            # only the actually-zero half-slab of each slab is cleared.
            def zrows(h_l):
                return slice(64, 128) if h_l % 2 == 0 else slice(0, 64)

            for h_l in (0, 1):
                nc.vector.memset(kt_sb[zrows(h_l), h_l, :], 0.0)
            for h_l in range(2, HL):
                nc.gpsimd.memset(kt_sb[zrows(h_l), h_l, :], 0.0)

            x_r = x_d.rearrange("(k p) n -> p k n", p=P)
            # Batched input DMA on the sync ring, ordered by first-use deadline; the
            # start is DMA-bandwidth-bound (8 cores pull at once), so fewer/bigger
            # triggers beat 59 per-chunk ones.
            nc.sync.dma_start(bqk_sb[:], bqk_d)
            nc.sync.dma_start(xt_sb[:, 0:4, 0:512], x_r[:, 0:4, 0:512])
            nc.sync.dma_start(wqk_sb[:, 0:4, 0:256], wqk_r[:, 0:4, 0:256])
            nc.sync.dma_start(xt_sb[:, 4:8, 0:512], x_r[:, 4:8, 0:512])
            nc.sync.dma_start(wqk_sb[:, 4:8, 0:256], wqk_r[:, 4:8, 0:256])
            nc.sync.dma_start(tri_sb[:], tri_d)
            nc.sync.dma_start(wv_sb[:, :, :], wv_r[:, :, :])
            nc.sync.dma_start(wqk_sb[:, :, 256:512], wqk_r[:, :, 256:512])
            nc.sync.dma_start(bv_sb[:], bv_d)
            nc.sync.dma_start(xt_sb[:, :, 512:1024], x_r[:, :, 512:1024])
            nc.sync.dma_start(wqk_sb[:, :, 512:768], wqk_r[:, :, 512:768])
            nc.sync.dma_start(wqk_sb[:, :, 768:1024], wqk_r[:, :, 768:1024])
            nc.sync.dma_start(xt_sb[:, :, 1024:1536], x_r[:, :, 1024:1536])
            nc.sync.dma_start(xt_sb[:, :, 1536:2048], x_r[:, :, 1536:2048])

            done_qk = set()
            done_v = set()

            def emit_qk(c, tt):
                if (c, tt) in done_qk:
                    return
                done_qk.add((c, tt))
                pr = c % 4
                pq = ps_mm.tile([P, 512], F32, tag="mm", name=f"pq_{c}_{tt}")
                col0 = 256 * (c % 4) + (0 if c < 4 else 128)
                for k in range(KC):
                    nc.tensor.matmul(
                        pq[:],
                        lhsT=wqk_sb[:, k, col0 : col0 + P],
                        rhs=xt_sb[:, k, tt * 512 : (tt + 1) * 512],
                        start=(k == 0),
                        stop=(k == KC - 1),
                    )
                def badd(out, in_, b):
                    nc.vector.tensor_scalar_add(out, in_, b)

                if c < 4:
                    badd(
                        qt_sb[:, pr, tt * 512 : (tt + 1) * 512], pq[:], bqk_sb[:, c : c + 1]
                    )
                else:
                    for hh in (0, 1):
                        rows = slice(64 * hh, 64 * hh + 64)
                        badd(
                            kt_sb[rows, 2 * pr + hh, tt * 512 : (tt + 1) * 512],
                            pq[rows, :],
                            bqk_sb[rows, c : c + 1],
                        )

            def emit_v(j, half):
                if (j, half) in done_v:
                    return
                done_v.add((j, half))
                pv = ps_mm.tile([P, VH], F32, tag="mm", name=f"pv_{j}_{half}")
                for k in range(KC):
                    nc.tensor.matmul(
                        pv[:],
                        lhsT=xt_sb[:, k, j * P : (j + 1) * P],
                        rhs=wv_sb[:, k, half * VH : (half + 1) * VH],
                        start=(k == 0),
                        stop=(k == KC - 1),
                    )
                nc.vector.tensor_add(
                    v_sb[:, j, half * VH : (half + 1) * VH],
                    pv[:],
                    bv_sb[:, half * VH : (half + 1) * VH],
                )

            filler = []
            for tt in range(4):
                for pr in range(4):
                    if (pr, tt) != (0, 0):
                        filler += [("qk", pr, tt), ("qk", pr + 4, tt)]
                filler += [("v", j, half) for j in range(4 * tt, 4 * tt + 4) for half in (0, 1)]
            state = {"i": 0}

            def pull(n):
                while n > 0 and state["i"] < len(filler):
                    it = filler[state["i"]]
                    state["i"] += 1
                    if it[0] == "v":
                        if (it[1], it[2]) in done_v:
                            continue
                        emit_v(it[1], it[2])
                    else:
                        if (it[1], it[2]) in done_qk:
                            continue
                        emit_qk(it[1], it[2])
                    n -= 1

            def emit_pv_half(p, t, pt, r, hh, ctx, split_dma=False):
                i = 4 * t + r
                if hh == 0:
                    ctx["opair"] = oppool.tile([P, P], F32, tag="op", name=f"op_{p}_{i}")
                    ctx["po"] = po = ps_o.tile([P, 2, 65], F32, tag="o", name=f"po_{p}_{i}")
                else:
                    po = ctx["po"]
                for j in range(i + 1):
                    nc.tensor.matmul(
                        po[:, hh, :],
                        lhsT=pt[:, hh, j, r * P : (r + 1) * P],
                        rhs=v_sb[:, j, 65 * (2 * p + hh) : 65 * (2 * p + hh) + 65],
                        start=(j == 0),
                        stop=(j == i),
                    )
                if hh == 0:
                    return
                opair = ctx["opair"]
                rc = mpool.tile([P, 2], F32, tag="rc", name=f"rc_{p}_{i}")
                nc.vector.reciprocal(rc[:], po[:, :, 64])
                for h2 in (0, 1):
                    if state.get("pos", 0) == 0:
                        nc.scalar.mul(
                            opair[:, 64 * h2 : 64 * h2 + 64],
                            po[:, h2, 0:64],
                            rc[:, h2 : h2 + 1],
                        )
                    else:
                        nc.vector.tensor_scalar_mul(
                            opair[:, 64 * h2 : 64 * h2 + 64], po[:, h2, 0:64], rc[:, h2 : h2 + 1]
                        )
                if split_dma:
                    # final blocks: trigger on the scalar HWDGE ring (idle after the
                    # last exp), avoiding the sync ring's DMA-sem-slot serialization
                    nc.scalar.dma_start(
                        out_d[i * P : (i + 1) * P, p * P : (p + 1) * P], opair[:]
                    )
                else:
                    nc.sync.dma_start(
                        out_d[i * P : (i + 1) * P, p * P : (p + 1) * P], opair[:]
                    )

            def emit_pv(p, t, pt, r, split_dma=False):
                ctx = {}
                emit_pv_half(p, t, pt, r, 0, ctx, split_dma)
                emit_pv_half(p, t, pt, r, 1, ctx, split_dma)

            # Prologue: the first block's QK tiles, emitted as k 0-3 / k 4-7 halves
            # interleaved across both tiles so each half tracks its DMA batch
            # (x stripe-0 and wqk pair-0 arrive as two 4-chunk batches).
            done_qk.update({(0, 0), (4, 0)})
            pq_pro = {
                0: ps_mm.tile([P, 512], F32, tag="mm", name="pq_pro_0"),
                4: ps_mm.tile([P, 512], F32, tag="mm", name="pq_pro_4"),
            }
            for part in (0, 1):
                for c in (0, 4):
                    col0 = 0 if c < 4 else 128
                    for k in range(4 * part, 4 * part + 4):
                        nc.tensor.matmul(
                            pq_pro[c][:],
                            lhsT=wqk_sb[:, k, col0 : col0 + P],
                            rhs=xt_sb[:, k, 0:512],
                            start=(k == 0),
                            stop=(k == KC - 1),
                        )
            nc.vector.tensor_scalar_add(qt_sb[:, 0, 0:512], pq_pro[0][:], bqk_sb[:, 0:1])
            for hh in (0, 1):
                rows = slice(64 * hh, 64 * hh + 64)
                nc.vector.tensor_scalar_add(
                    kt_sb[rows, hh, 0:512], pq_pro[4][rows, :], bqk_sb[rows, 4:5]
                )

            pv_queue = []
            blocks = [(pos, t, p) for pos, t in enumerate((0, 1, 2, 3)) for p in range(4)]
            for n, (pos, t, p) in enumerate(blocks):
                    state["pos"] = pos
                    last = n == len(blocks) - 1
                    for tt in range(t + 1):
                        emit_qk(p, tt)
                        emit_qk(4 + p, tt)
                    nxt_qk = []
                    if n + 1 < len(blocks):
                        _, tn, pn = blocks[n + 1]
                        nxt_qk = [
                            (c, tt)
                            for tt in range(tn + 1)
                            for c in (pn, 4 + pn)
                            if (c, tt) not in done_qk
                        ]
                    pt = ptpool.tile([P, 2, 16, 512], F16, tag="pt", name=f"pt_{p}_{t}")
                    vpend = [
                        (j, half)
                        for j in range(4 * t, 4 * t + 4)
                        for half in (0, 1)
                        if (j, half) not in done_v
                    ]

                    def group_hooks(pos=pos, vpend=vpend, nxt_qk=nxt_qk):
                        if pv_queue:
                            emit_pv(*pv_queue.pop(0))
                        if vpend:
                            emit_v(*vpend.pop(0))
                        if nxt_qk:
                            emit_qk(*nxt_qk.pop(0))
                            return
                        state["g"] = state.get("g", 0) + 1
                        if pos == 3:
                            pull(1)

                    for g in range(2 * t + 2):
                        psA = ps_s.tile([P, 2, 512], F32, tag="s", name=f"psA_{p}_{t}_{g}")
                        psB = ps_s.tile([P, 2, 512], F32, tag="s", name=f"psB_{p}_{t}_{g}")
                        for jj in (0, 1):
                            j = 2 * g + jj
                            q0 = 128 * (j - 4 * t) if j >= 4 * t else 0
                            for hh, ps in ((0, psA), (1, psB)):
                                nc.tensor.matmul(
                                    ps[:, jj, q0:512],
                                    lhsT=kt_sb[:, 2 * p + hh, j * P : (j + 1) * P],
                                    rhs=qt_sb[:, p, t * 512 + q0 : (t + 1) * 512],
                                    start=True,
                                    stop=True,
                                )
                        for hh, ps in ((0, psA), (1, psB)):
                            if g == 2 * t + 1:
                                # one act over both chunks' 256:512 suffix (chunk
                                # 2g+1's 256:384 is stale finite psum, exp'd but
                                # never consumed; cheaper than a second act)
                                nc.scalar.activation(
                                    pt[:, hh, 2 * g : 2 * g + 2, 256:512],
                                    ps[:, :, 256:512],
                                    mybir.ActivationFunctionType.Exp,
                                    scale=0.125,
                                )
                            else:
                                nc.scalar.activation(
                                    pt[:, hh, 2 * g : 2 * g + 2, :],
                                    ps[:],
                                    mybir.ActivationFunctionType.Exp,
                                    scale=0.125,
                                )
                        group_hooks()
                        if last and g >= 2 * t:
                            for r in (0, 1) if g == 2 * t else (2, 3):
                                j = 4 * t + r
                                for hh in (0, 1):
                                    blk = pt[:, hh, j, r * P : (r + 1) * P]
                                    nc.vector.tensor_mul(blk, blk, tri_sb[:])
                                emit_pv(p, t, pt, r, split_dma=(g == 2 * t + 1))
                    while pv_queue:
                        emit_pv(*pv_queue.pop(0))
                    if last:
                        continue
                    for hh in (0, 1):
                        for r in range(4):
                            j = 4 * t + r
                            blk = pt[:, hh, j, r * P : (r + 1) * P]
                            nc.vector.tensor_mul(blk, blk, tri_sb[:])
                    for j in range(4 * t + 4):
                        emit_v(j, p // 2)
                    pv_queue = [(p, t, pt, r) for r in range(4)]
            while pv_queue:
                emit_pv(*pv_queue.pop(0))
            pull(len(filler))  # safety: flush

    nc.compile()
    return nc


def get_nc():
    if "nc" not in _cache:
        _cache["nc"] = _build()
    return _cache["nc"]


def _prep_core_inputs(x, W, b, bi, hg):
    h0 = hg * HL
    Wq = W[:, 0:D].reshape(D, H, HD)
    Wk = W[:, D : 2 * D].reshape(D, H, HD)
    Wv = W[:, 2 * D :].reshape(D, H, HD)
    bq = b[0:D].reshape(H, HD)
    bk = b[D : 2 * D].reshape(H, HD)
    bv = b[2 * D :].reshape(H, HD)

    wqk = np.empty((D, 1024), np.float32)
    bqk = np.empty((P, 8), np.float32)
    for c in range(4):
        for half in range(2):
            h = h0 + 2 * c + half
            sl = slice(256 * c + half * HD, 256 * c + half * HD + HD)
            wqk[:, sl] = Wq[:, h]
            bqk[half * HD : (half + 1) * HD, c] = bq[h]
            sl = slice(256 * c + P + half * HD, 256 * c + P + half * HD + HD)
            wqk[:, sl] = Wk[:, h]
            bqk[half * HD : (half + 1) * HD, 4 + c] = bk[h]

    wv_aug = np.zeros((D, VW), np.float32)
    bv_aug = np.zeros((VW,), np.float32)
    for hl in range(HL):
        wv_aug[:, 65 * hl : 65 * hl + HD] = Wv[:, h0 + hl]
        bv_aug[65 * hl : 65 * hl + HD] = bv[h0 + hl]
        bv_aug[65 * hl + HD] = 1.0

    tri = np.triu(np.ones((P, P), np.float32))

    return {
        "x": np.ascontiguousarray(x[bi].astype(np.float16).T),
        "wqk": wqk.astype(np.float16),
        "wv": wv_aug.astype(np.float16),
        "bqk": bqk,
        "bv": np.broadcast_to(bv_aug.astype(np.float16), (P, VW)).copy(),
        "tri": tri.astype(np.float16),
    }


def make_in_maps(x, W_qkv, b_qkv):
    x = np.asarray(x, dtype=np.float32)
    W = np.asarray(W_qkv, dtype=np.float32)
    b = np.asarray(b_qkv, dtype=np.float32)
    return [_prep_core_inputs(x, W, b, i // 2, i % 2) for i in range(N_CORES)]


def assemble(results):
    out = np.empty((B, N, D), np.float32)
    for i in range(N_CORES):
        bi, hg = i // 2, i % 2
        out[bi, :, hg * 512 : (hg + 1) * 512] = results[i]["out"]
    return out


def run(x, W_qkv, b_qkv, trace=False, tmpdir=None):
    nc = get_nc()
    in_maps = make_in_maps(x, W_qkv, b_qkv)
    res = bass_utils.run_bass_kernel_spmd(
        nc, in_maps, core_ids=list(range(N_CORES)), trace=trace, tmpdir=tmpdir
    )
    return assemble(res.results), res


def kernel(x, W_qkv, b_qkv):
    out, _ = run(x, W_qkv, b_qkv)
    return out
